# revision 6
# baseline (speedup 1.0000x reference)
"""Trainium2 Bass kernel for PVT-style spatial-reduction attention.

Model (see reference):
  q = (x @ Wq + bq) * hd^-0.5                       (B, N, C) -> heads of 32
  x_ = BN(DWConv2x2s2(x)) ; k = x_ @ Wk + bk ; v = x_ @ Wv + bv
  attn = softmax(q k^T + rel_pos) ; out = (attn @ v) @ Wp + bp

Shapes: B=8, N=3136 (56x56), C=128, heads=4, hd=32, Nkv=784 (28x28).

Distribution: each of 8 cores handles a slice of 392 query rows (N/8) for
ALL batches and heads.  rel_pos then splits exactly 8 ways and each core
produces final output rows locally (no cross-core reduction).

Device layout strategy: features-on-partitions everywhere (C == 128).
  - host does the (tiny) depthwise 2x2/s2 conv + BN fold and ships the
    spatially-reduced activation transposed, xcT (B, C, Nkv) bf16, plus
    the query slice xTn (B, C, NSL).  All heavy lifting (q/k/v
    projections, scores, softmax exp, attn@v, out proj) stays on device.
  - kT = Wk^T-matmul(xcT) in two 392-col matmuls (full 128 contraction);
    k-bias dropped (softmax-invariant).  v is produced directly in
    [m-part, c] layout by using xcT chunks as the stationary operand
    (lhsT) against Wv: out[m, c], 7 matmuls, no separate transpose.
    v-bias folded into the final bias.  Both write bf16 PSUM and are
    copied out by DMA (no Vector/Scalar copy cost).
  - scores computed transposed: S^T[m, n] per (b, h); softmax uses
    exp(S + R) = exp(S) * exp(R) with exp(rel_pos^T) precomputed on host.
    No max-subtraction (|S| < 1 by construction).
  - row sums ride as a ones-column appended to v in the attn@v matmul;
    normalization is a block-broadcast matmul + reciprocal_approx_fast.
  - PE array tiling: the 4 heads' score matmuls run on 4 concurrent
    32x128 row-tiles; attn@v runs head pairs on two col-tiles {0, 64}.
  - the exp(S)*expR multiplies alternate between VectorE (head pair 0)
    and GpSimd (head pair 1, which has more slack) to split the
    elementwise load across engines.
  - emission runs on a global 14-step-per-batch clock with the
    score/exp stream TWO steps ahead of the attn@v/prep stream, so the
    ScalarE exp pipeline never stalls at batch boundaries (score
    matmuls enter the PE FIFO before the batch-tail attn@v work).
  - final output is produced transposed (B, C, NSL); the host gather
    untransposes while assembling the full (B, N, C) result.
"""

import os
import sys

import numpy as np

if "/opt/trn_rl_repo" not in sys.path:
    sys.path.insert(0, "/opt/trn_rl_repo")

B = 8
N = 3136
C = 128
HEADS = 4
HD = 32
SR = 2
H = W = 56
NKV = 784  # 28*28
NCORES = 8
NSL = N // NCORES  # 392 query rows per core
BN_EPS = 1e-5
SCALE = HD ** -0.5

# m (kv index) chunking: 784 = 6*128 + 16
M_CHUNKS = [(j * 128, min(128, NKV - j * 128)) for j in range((NKV + 127) // 128)]

PROB_BF16 = os.environ.get("KERNEL_PROB_BF16", "1") == "1"

_COMPILED = None  # cached (nc, meta) across kernel() calls


def _host_prep(x, relative_pos, Wq, bq, Wk, bk, Wv, bv, conv_w, conv_b,
               bn_gamma, bn_beta, bn_mean, bn_var, Wp, bp):
    """Fold conv/BN on host; transpose activations; fold biases."""
    import ml_dtypes
    f32 = np.float32
    bf16 = ml_dtypes.bfloat16
    wdt = bf16 if PROB_BF16 else f32
    x = np.asarray(x, f32)
    # xTn: (B, C, N) -> per-core (B, C, NSL) query slices, sliced in make_in_map
    xT = np.ascontiguousarray(x.transpose(0, 2, 1).astype(wdt))

    # depthwise conv 2x2 s2 + BN (eval) on host -> xcT (B, C, NKV)
    inv = (np.asarray(bn_gamma, f32)
           / np.sqrt(np.asarray(bn_var, f32) + BN_EPS))          # [c]
    wp_taps = np.asarray(conv_w, f32).reshape(C, SR * SR) * inv[:, None]  # [c,4]
    beta0 = (np.asarray(conv_b, f32) * inv
             + np.asarray(bn_beta, f32)
             - np.asarray(bn_mean, f32) * inv)                    # [c]
    x_img = x.transpose(0, 2, 1).reshape(B, C, H, W)
    y = np.zeros((B, C, H // SR, W // SR), f32)
    for t in range(SR * SR):
        di, dj = t // 2, t % 2
        y += x_img[:, :, di::SR, dj::SR] * wp_taps[None, :, t, None, None]
    y += beta0[None, :, None, None]
    xcT = np.ascontiguousarray(y.reshape(B, C, NKV).astype(wdt))

    Wk_s = np.ascontiguousarray(np.asarray(Wk, f32).astype(wdt))
    Wv_s = np.ascontiguousarray(np.asarray(Wv, f32).astype(wdt))

    # v bias (uniform over kv positions -> exact fold into final bias)
    beta_v = np.asarray(bv, f32)                                  # [c']
    bp_col = (np.asarray(bp, f32) + beta_v @ np.asarray(Wp, f32)).reshape(C, 1)

    Wq_s = np.ascontiguousarray((np.asarray(Wq, f32) * SCALE).astype(wdt))
    bq_col = (np.asarray(bq, f32) * SCALE).reshape(C, 1)

    # exp(rel)^T per core: (4, NKV, NSL)
    rel = np.asarray(relative_pos, f32)
    expRT = []
    for j in range(NCORES):
        sl = rel[:, j * NSL:(j + 1) * NSL, :]          # (4, NSL, NKV)
        e = np.exp(sl).transpose(0, 2, 1)              # (4, NKV, NSL)
        if PROB_BF16:
            e = e.astype(bf16)
        expRT.append(np.ascontiguousarray(e))

    emat = np.zeros((HEADS, C), f32)
    for h in range(HEADS):
        emat[h, HD * h:HD * (h + 1)] = 1.0

    return dict(emat=emat,
                xT=xT, xcT=xcT, Wk=Wk_s, Wv=Wv_s, Wq=Wq_s, bq=bq_col,
                Wp=np.ascontiguousarray(np.asarray(Wp, f32)), bp=bp_col,
                expRT=expRT)


def _build():
    """Build + compile the SPMD bass program (same NEFF for all 8 cores)."""
    import concourse.bass as bass
    import concourse.tile as tile
    from concourse import bacc, mybir

    f32 = mybir.dt.float32
    f32r = mybir.dt.float32r
    pdt = mybir.dt.bfloat16 if PROB_BF16 else f32

    nc = bacc.Bacc("TRN2", target_bir_lowering=False, debug=False,
                   num_devices=NCORES)

    # ---- DRAM I/O ----
    xcT_d = nc.dram_tensor("xcT", [B, C, NKV], pdt, kind="ExternalInput").ap()
    xTn_d = nc.dram_tensor("xTn", [B, C, NSL], pdt, kind="ExternalInput").ap()
    expRT_d = nc.dram_tensor("expRT", [HEADS, NKV, NSL],
                             pdt, kind="ExternalInput").ap()
    Wq_d = nc.dram_tensor("Wq", [C, C], pdt, kind="ExternalInput").ap()
    bq_d = nc.dram_tensor("bq", [C, 1], f32, kind="ExternalInput").ap()
    Wk_d = nc.dram_tensor("Wk", [C, C], pdt, kind="ExternalInput").ap()
    Wv_d = nc.dram_tensor("Wv", [C, C], pdt, kind="ExternalInput").ap()
    Wp_d = nc.dram_tensor("Wp", [C, C], f32r, kind="ExternalInput").ap()
    bp_d = nc.dram_tensor("bp", [C, 1], f32, kind="ExternalInput").ap()
    emat_d = nc.dram_tensor("emat", [HEADS, C], f32r, kind="ExternalInput").ap()
    out_d = nc.dram_tensor("out", [B, C, NSL], f32, kind="ExternalOutput").ap()

    with tile.TileContext(nc) as tc:
        from contextlib import ExitStack
        with ExitStack() as ctx:
            _emit(ctx, tc, nc, bass, mybir, f32, f32r, pdt,
                  xcT_d, xTn_d, expRT_d, Wq_d, bq_d, Wk_d, Wv_d,
                  Wp_d, bp_d, emat_d, out_d)

    nc.compile()
    return nc


def _emit(ctx, tc, nc, bass, mybir, f32, f32r, pdt,
          xcT_d, xTn_d, expRT_d, Wq_d, bq_d, Wk_d, Wv_d,
          Wp_d, bp_d, emat_d, out_d):
    AF = mybir.ActivationFunctionType

    singles = ctx.enter_context(tc.tile_pool(name="singles", bufs=1))
    xpool = ctx.enter_context(tc.tile_pool(name="xpool", bufs=3))
    qkv = ctx.enter_context(tc.tile_pool(name="qkv", bufs=3))
    ppool = ctx.enter_context(tc.tile_pool(name="ppool", bufs=3))
    opool = ctx.enter_context(tc.tile_pool(name="opool", bufs=3))
    vpool = ctx.enter_context(tc.tile_pool(name="vpool", bufs=3))
    ptpool = ctx.enter_context(tc.tile_pool(name="ptpool", bufs=6))
    ps_small = ctx.enter_context(tc.tile_pool(name="ps_small", bufs=2,
                                              space="PSUM"))
    ps_scoA = ctx.enter_context(tc.tile_pool(name="ps_scoA", bufs=1,
                                             space="PSUM"))
    ps_scoB = ctx.enter_context(tc.tile_pool(name="ps_scoB", bufs=1,
                                             space="PSUM"))
    ps_o = ctx.enter_context(tc.tile_pool(name="ps_o", bufs=1, space="PSUM"))

    # ---- constants ----
    emat_sb = singles.tile([HEADS, C], f32r)
    nc.sync.dma_start(out=emat_sb[:], in_=emat_d)
    wq_sb = singles.tile([C, C], pdt)
    nc.sync.dma_start(out=wq_sb[:], in_=Wq_d)
    bq_sb = singles.tile([C, 1], f32)
    nc.sync.dma_start(out=bq_sb[:], in_=bq_d)
    wk_sb = singles.tile([C, C], pdt)
    nc.sync.dma_start(out=wk_sb[:], in_=Wk_d)
    wv_sb = singles.tile([C, C], pdt)
    nc.sync.dma_start(out=wv_sb[:], in_=Wv_d)
    wp_sb = singles.tile([C, C], f32r)
    nc.sync.dma_start(out=wp_sb[:], in_=Wp_d)
    bp_sb = singles.tile([C, 1], f32)
    nc.sync.dma_start(out=bp_sb[:], in_=bp_d)

    # expRT interleaved: [128, 7 chunks, 4 heads, 392]
    expTI = singles.tile([C, 7, HEADS, NSL], pdt)
    nc.vector.memset(expTI[:, 6, :, :], 0.0)
    for h in range(HEADS):
        src = expRT_d[h]  # (784, 392)
        nc.sync.dma_start(
            out=expTI[:, 0:6, h, :],
            in_=src[0:768].rearrange("(j p) i -> p j i", p=128))
        nc.sync.dma_start(out=expTI[0:16, 6, h, :], in_=src[768:784])

    state = {}
    pp_of = {}

    def prep_load(b):
        s = state.setdefault(b, {})
        xcT_sb = xpool.tile([C, NKV], pdt, tag="xcT")
        s["xcT"] = xcT_sb
        nc.sync.dma_start(out=xcT_sb[:], in_=xcT_d[b])
        xTn_sb = xpool.tile([C, NSL], pdt, tag="xTn")
        s["xTn"] = xTn_sb
        nc.sync.dma_start(out=xTn_sb[:], in_=xTn_d[b])

    def prep_k(b):
        """kT = Wk^T @ xcT: two 392-col matmuls, Vector-copied to bf16."""
        s = state[b]
        kT_sb = qkv.tile([C, 7 * 128], pdt, tag="kT")
        s["kT"] = kT_sb
        nc.vector.memset(kT_sb[:, NKV:7 * 128], 0.0)
        for half in range(2):
            ps_k = ps_small.tile([C, 512], f32, tag="small")
            nc.tensor.matmul(ps_k[:, 0:392], lhsT=wk_sb[:],
                             rhs=s["xcT"][:, 392 * half:392 * (half + 1)],
                             start=True, stop=True)
            nc.vector.tensor_copy(kT_sb[:, 392 * half:392 * (half + 1)],
                                  ps_k[:, 0:392])

    def prep_v(b):
        """v[m, c] per kv-chunk: xcT chunk as stationary operand vs Wv."""
        s = state[b]
        v_sb = vpool.tile([C, 7, HEADS, HD + 1], pdt, tag="v")
        s["v"] = v_sb
        nc.vector.memset(v_sb[:, :, :, HD:HD + 1], 1.0)
        ps_va = ps_small.tile([C, 4, C], f32, tag="small")
        for j in range(4):
            nc.tensor.matmul(ps_va[:, j, :],
                             lhsT=s["xcT"][:, 128 * j:128 * (j + 1)],
                             rhs=wv_sb[:], start=True, stop=True)
        nc.vector.tensor_copy(
            v_sb[:, 0:4, :, 0:HD],
            ps_va[:].rearrange("p j (h d) -> p j h d", h=HEADS, d=HD))
        ps_vb = ps_small.tile([C, 3, C], f32, tag="small")
        for j in range(4, 7):
            m0, cnt = M_CHUNKS[j]
            nc.tensor.matmul(ps_vb[0:cnt, j - 4, :],
                             lhsT=s["xcT"][:, m0:m0 + cnt], rhs=wv_sb[:],
                             start=True, stop=True)
        nc.vector.tensor_copy(
            v_sb[:, 4:6, :, 0:HD],
            ps_vb[:, 0:2, :].rearrange("p j (h d) -> p j h d", h=HEADS, d=HD))
        nc.vector.tensor_copy(
            v_sb[0:16, 6, :, 0:HD],
            ps_vb[0:16, 2, :].rearrange("p (h d) -> p h d", h=HEADS, d=HD))

    def prep_q(b):
        s = state[b]
        ps_q = ps_small.tile([C, 512], f32, tag="small")
        nc.tensor.matmul(ps_q[:, 0:NSL], lhsT=wq_sb[:], rhs=s.pop("xTn")[:],
                         start=True, stop=True)
        qT_sb = qkv.tile([C, NSL], pdt, tag="qT")
        s["qT"] = qT_sb
        nc.vector.tensor_scalar_add(qT_sb[:], ps_q[:, 0:NSL], bq_sb[:, 0:1])

    def half_round(b, r, hp):
        """Scores + exp + expR multiply for chunk r, head pair hp."""
        s = state[b]
        pool = ps_scoA if hp == 0 else ps_scoB
        ps_s = pool.tile([C, 2, 512], f32, tag="sco%d" % hp)
        for hh in range(2):
            h = 2 * hp + hh
            nc.tensor.matmul(
                ps_s[0:128, hh, 0:NSL],
                lhsT=s["kT"][HD * h:HD * (h + 1), 128 * r:128 * (r + 1)],
                rhs=s["qT"][HD * h:HD * (h + 1), :],
                start=True, stop=True,
                tile_position=(HD * h, 0))
        pt_sb = ptpool.tile([C, 2, NSL], pdt, tag="pt")
        nc.scalar.activation(pt_sb[:], ps_s[:, :, 0:NSL], AF.Exp)
        eng = nc.vector if (hp == 0 and r <= 3) else nc.gpsimd
        eng.tensor_mul(pp_of[b][:, 2 * hp:2 * hp + 2, r, :], pt_sb[:],
                       expTI[:, r, 2 * hp:2 * hp + 2, :])

    def attnv_chunk(b, hp, rr):
        """attn@v accumulation for head pair hp over kv chunks rr."""
        s = state[b]
        if 0 in rr:
            ps_ov = ps_o.tile([C, 2, 512], f32, tag="ov")
            s["ov%d" % hp] = ps_ov
        else:
            ps_ov = s["ov%d" % hp]
        for r in rr:
            m0, cnt = M_CHUNKS[r]
            for hh in range(2):
                h = 2 * hp + hh
                nc.tensor.matmul(
                    ps_ov[64 * hh:64 * hh + HD + 1, hh, 0:NSL],
                    lhsT=s["v"][0:cnt, r, h, :],
                    rhs=pp_of[b][0:cnt, h, r, :],
                    start=(r == 0), stop=(r == len(M_CHUNKS) - 1),
                    tile_position=(0, 64 * hh))

    def attnv_extract(b, hp):
        """Vector-copy row-sums + head outputs out of the PSUM accumulator."""
        s = state[b]
        ps_ov = s.pop("ov%d" % hp)
        if "rs" not in s:
            rs_t = opool.tile([1, HEADS * NSL], f32r, tag="rs")
            outTr_t = opool.tile([C, NSL], f32, tag="outTr")
            s["rs"], s["outTr"] = rs_t, outTr_t
        for hh in range(2):
            h = 2 * hp + hh
            nc.vector.tensor_copy(s["rs"][0:1, NSL * h:NSL * (h + 1)],
                                  ps_ov[64 * hh + HD:64 * hh + HD + 1,
                                        hh, 0:NSL])
            nc.vector.tensor_copy(s["outTr"][HD * h:HD * (h + 1), :],
                                  ps_ov[64 * hh:64 * hh + HD, hh, 0:NSL])

    def norm(b):
        """rowsums -> 4 partitions -> block broadcast -> recip -> multiply."""
        s = state[b]
        rs4_sb = opool.tile([HEADS, NSL], f32r, tag="rs4")
        nc.sync.dma_start(
            out=rs4_sb[:],
            in_=s.pop("rs")[0:1, :].rearrange("p (h i) -> p h i", h=HEADS))
        ps_rb = ps_small.tile([C, 512], f32, tag="small")
        nc.tensor.matmul(ps_rb[:, 0:NSL], lhsT=emat_sb[:], rhs=rs4_sb[:],
                         start=True, stop=True)
        rb_sb = opool.tile([C, NSL], f32, tag="rb")
        nc.vector.reciprocal_approx_fast(rb_sb[:], ps_rb[:, 0:NSL])
        outT_sb = opool.tile([C, NSL], f32r, tag="outT")
        s["outT"] = outT_sb
        nc.vector.tensor_mul(outT_sb[:], s.pop("outTr")[:], rb_sb[:])

    def proj_tail(b):
        """Final projection in transposed layout; host untransposes."""
        s = state[b]
        ps_ft = ps_small.tile([C, 512], f32, tag="small")
        nc.tensor.matmul(ps_ft[:, 0:NSL], lhsT=wp_sb[:], rhs=s.pop("outT")[:],
                         start=True, stop=True)
        fin_sb = opool.tile([C, NSL], f32, tag="fin")
        nc.vector.tensor_scalar_add(fin_sb[:], ps_ft[:, 0:NSL],
                                    bp_sb[:, 0:1])
        nc.sync.dma_start(out=out_d[b], in_=fin_sb[:])
        state.pop(b)

    def score_step(sg):
        bs, ss = divmod(sg, 14)
        if ss == 0:
            pp_of[bs] = ppool.tile([C, HEADS, 7, NSL], pdt, tag="pp",
                                   name="pp")
            pp_of.pop(bs - 3, None)
        half_round(bs, ss // 2, ss % 2)

    # ---- prologue ----
    prep_load(0)
    prep_k(0)
    prep_v(0)
    prep_q(0)
    prep_load(1)
    score_step(0)
    score_step(1)

    # ---- global pipeline: score stream runs 2 steps ahead ----
    for g in range(B * 14):
        b, step = divmod(g, 14)
        if g + 2 < B * 14:
            score_step(g + 2)
        if step == 0 and b + 2 < B:
            prep_load(b + 2)
        if b >= 1:
            if step == 0:
                attnv_chunk(b - 1, 1, (0, 1))
            elif step == 2:
                attnv_chunk(b - 1, 1, (2, 3))
            elif step == 4:
                attnv_chunk(b - 1, 1, (4, 5, 6))
            elif step == 5:
                attnv_extract(b - 1, 1)
            elif step == 6:
                norm(b - 1)
            elif step == 7:
                proj_tail(b - 1)
        if b + 1 < B:
            if step == 2:
                prep_k(b + 1)
            elif step == 3:
                prep_v(b + 1)
            elif step == 4:
                prep_q(b + 1)
        if step == 7:
            attnv_chunk(b, 0, (0, 1))
        elif step == 9:
            attnv_chunk(b, 0, (2, 3))
        elif step == 11:
            attnv_chunk(b, 0, (4, 5))
        elif step == 13:
            attnv_chunk(b, 0, (6,))
            attnv_extract(b, 0)

    # ---- epilogue: last batch's second head pair ----
    attnv_chunk(B - 1, 1, (0, 1))
    attnv_chunk(B - 1, 1, (2, 3))
    attnv_chunk(B - 1, 1, (4, 5, 6))
    attnv_extract(B - 1, 1)
    norm(B - 1)
    proj_tail(B - 1)


def _get_compiled():
    global _COMPILED
    if _COMPILED is None:
        _COMPILED = _build()
    return _COMPILED


def make_in_map(prep, j):
    return {
        "xcT": prep["xcT"],
        "xTn": np.ascontiguousarray(prep["xT"][:, :, j * NSL:(j + 1) * NSL]),
        "expRT": prep["expRT"][j],
        "Wq": prep["Wq"], "bq": prep["bq"],
        "Wk": prep["Wk"], "Wv": prep["Wv"],
        "Wp": prep["Wp"], "bp": prep["bp"], "emat": prep["emat"],
    }


def kernel(x, relative_pos, Wq, bq, Wk, bk, Wv, bv, conv_w, conv_b,
           bn_gamma, bn_beta, bn_mean, bn_var, Wp, bp, H=56, W=56,
           _trace=False):
    from concourse.bass_utils import run_bass_kernel_spmd

    prep = _host_prep(x, relative_pos, Wq, bq, Wk, bk, Wv, bv, conv_w,
                      conv_b, bn_gamma, bn_beta, bn_mean, bn_var, Wp, bp)
    nc = _get_compiled()

    in_maps = [make_in_map(prep, j) for j in range(NCORES)]

    res = run_bass_kernel_spmd(nc, in_maps, core_ids=list(range(NCORES)),
                               trace=_trace)

    out = np.empty((B, N, C), np.float32)
    for j in range(NCORES):
        out[:, j * NSL:(j + 1) * NSL, :] = \
            res.results[j]["out"].transpose(0, 2, 1)
    if _trace:
        kernel._last_result = res
    return out
